# revision 32
# baseline (speedup 1.0000x reference)
"""CMSBlockLinear block-ELL sparse linear forward on 8 trn2 NeuronCores.

Strategy: the block-sparse weight (R=128 x K=32 active 16x16 tiles, 25%
density) is densified on the host into W^T [2048 in, 2048 out] and cast to
bf16.  The device then runs a dense matmul y^T = W^T.T @ x^T with fp32 PSUM
accumulation.  Dense-ifying costs 4x the weight FLOPs on paper, but the PE
streams N columns per matmul regardless of M, so a dense 128-wide M uses the
array 8x better than the natural M=16 sparse formulation - dense wins on both
PE time and (with bf16) roughly matches sparse fp32 on DMA bytes.

Sharding (8 cores): 4-way over tokens x 2-way over output features.
Per core: x^T shard [2048, 512] bf16 (2 MB), W^T half [2048, 1024] bf16
(4 MB), out [1024, 512] bf16 (1 MB, upcast on host).

Device loop: k-outer over 16 contraction chunks of 128; chunks alternate
between the two HWDGE rings (Sync and Scalar) so the aggregate input
bandwidth (~270 GB/s) comfortably exceeds the PE's 150 GB/s demand.  8
dummy matmuls at the top hold the PE's HAM clock through the first-chunk
DMA wait; all 8 psum->sbuf copies ride DVE (no scalar ACT, so Scalar's
first descriptor push isn't blocked behind ACT_TABLE_LOAD); output DMAs
are pushed per-m as each copy completes, alternating rings, with the
last bank's epilogue matmuls/copies/DMAs split into token-halves so the
final 64 KB transfers' completion sems gate the exit barrier as early
as possible; the very last half-copy rides ACT (its only instruction,
so nothing queues ahead of it) in parallel with DVE's second-to-last.

Measured ~44.5-45.3 us (run-to-run spread ~0.5 us from walrus-preamble
and HAM-window phase; best observed 44522 ns).  Breakdown (from NTFF profile): the measured
window runs from the framework's const-pool memsets (~5.9 us, walrus
preamble before that is excluded) to the last exit-ladder instruction.
Fixed costs inside the window: ~1 us of preamble tail + a ~7.4 us walrus
exit ladder (each engine serially re-checks ~57 semaphores at NEFF exit;
pace is dispatch-bound — overlapping it with tail DMAs makes it slower,
measured).  The PE stream itself is ~27.3 us at the bf16 roofline
(65536 col-streams @ 2.4 GHz), starting ~11 us (first-chunk DMA
descriptor-gen + completion-sem latency) with ~0.8 us of HAM cold-clock
penalty.  fp8 (e4m3 both operands, DoubleRow) would cut PE time 1.44x
but measures 3.8% output error vs the 2% gate; int8 would pass (1.3%)
but the BIR verifier rejects integer Matmult dtypes on this toolchain.
"""

import os

import numpy as np

BATCH, SEQ = 4, 512
IN_F = OUT_F = 2048
B = 16
R = 128  # output block rows
C = 128  # input block cols
KBLK = 32  # active tiles per row

TOK = BATCH * SEQ  # 2048 tokens
TOK_SHARDS = 4
OUT_SHARDS = 2
TOK_PER = TOK // TOK_SHARDS  # 512
OUT_PER = OUT_F // OUT_SHARDS  # 1024
K_CHUNKS = IN_F // 128  # 16
M_CHUNKS = OUT_PER // 128  # 8

LAST_EXEC_TIME_NS = None

_CACHE = {}


def _ensure_profile_hook():
    """Provide antenv.axon_hooks if the image lacks it, so trace=True works.

    Mirrors trn_agent_boot._ntff_profile_via_ctypes: drives NTFF capture via
    the libaxon_pjrt.so C ABI.  Also makes upload_artifacts fall back to the
    local dir when no artifact store is reachable.
    """
    import contextlib
    import ctypes
    import sys
    import types

    try:
        import antenv.axon_hooks  # noqa: F401

        return
    except ImportError:
        pass

    so_path = "/opt/axon/libaxon_pjrt.so"
    _hook = None
    if os.path.exists(so_path):
        try:
            lib = ctypes.CDLL(so_path)
            if hasattr(lib, "axon_start_nrt_profile"):
                lib.axon_start_nrt_profile.argtypes = [
                    ctypes.POINTER(ctypes.c_int64),
                    ctypes.c_size_t,
                ]
                lib.axon_start_nrt_profile.restype = ctypes.c_int64
                lib.axon_stop_nrt_profile.argtypes = [ctypes.c_char_p]
                lib.axon_stop_nrt_profile.restype = ctypes.c_int64

                @contextlib.contextmanager
                def _ntff_hook(output_dir, device_ids):
                    import jax

                    jax.devices()
                    if device_ids:
                        ids = (ctypes.c_int64 * len(device_ids))(*device_ids)
                        rc = lib.axon_start_nrt_profile(ids, len(device_ids))
                    else:
                        rc = lib.axon_start_nrt_profile(None, 0)
                    if rc != 0:
                        raise RuntimeError(f"axon_start_nrt_profile rc={rc}")
                    try:
                        yield
                    finally:
                        n = lib.axon_stop_nrt_profile(str(output_dir).encode())
                        print(f"profile: {n} file(s) -> {output_dir}", file=sys.stderr)

                _hook = _ntff_hook
        except OSError:
            pass

    mod = types.ModuleType("antenv.axon_hooks")
    mod.get_axon_ntff_profile_hook = lambda: _hook
    sys.modules["antenv.axon_hooks"] = mod

    import concourse.bass_utils as _bu

    _orig_upload = _bu.upload_artifacts

    def _safe_upload(tmpdir):
        try:
            return _orig_upload(tmpdir)
        except Exception:
            return tmpdir

    _bu.upload_artifacts = _safe_upload


def _build_nc():
    import concourse.mybir as mybir
    from concourse import bacc
    from concourse.tile import TileContext

    nc = bacc.Bacc("TRN2", target_bir_lowering=False)
    xT = nc.dram_tensor("xT", [IN_F, TOK_PER], mybir.dt.bfloat16, kind="ExternalInput")
    w = nc.dram_tensor("w", [IN_F, OUT_PER], mybir.dt.bfloat16, kind="ExternalInput")
    bias = nc.dram_tensor("bias", [OUT_PER], mybir.dt.float32, kind="ExternalInput")
    # y device layout: [partition, m-chunk, token] in m order; host un-permutes.
    y = nc.dram_tensor(
        "y", [128, M_CHUNKS * TOK_PER], mybir.dt.bfloat16, kind="ExternalOutput"
    )

    with TileContext(nc) as tc:
        with (
            tc.tile_pool(name="consts", bufs=1) as consts,
            tc.tile_pool(name="xp", bufs=K_CHUNKS) as xp,
            tc.tile_pool(name="wp", bufs=K_CHUNKS) as wp,
            tc.tile_pool(name="op", bufs=1) as op,
            tc.tile_pool(name="ps", bufs=1, space="PSUM") as ps,
        ):
            psums = [
                ps.tile([128, TOK_PER], mybir.dt.float32, tag=f"ps{m}", name=f"ps{m}")
                for m in range(M_CHUNKS)
            ]

            # HAM warm-up: dummy matmuls bridge the entry barrier (~6.9us)
            # to first-chunk data arrival (~10.3us) so the PE's activity
            # window is already hot when the real stream starts.
            warm = consts.tile([128, TOK_PER], mybir.dt.bfloat16)
            nc.vector.memset(warm[:, :1], 0)
            N_WARM = 8
            for i in range(N_WARM):
                nc.tensor.matmul(
                    psums[0][:],
                    warm[:, :128],
                    warm[:],
                    start=(i == 0),
                    stop=(i == N_WARM - 1),
                )

            # Both inputs ride both HWDGE rings: even w chunks + odd x
            # chunks on Scalar, odd w chunks + even x chunks on Sync.  Each
            # ring then carries ~(128+64) KB per 1.7us chunk period
            # (~113 GB/s), under the ~134 GB/s per-ring ceiling.  Chunk 0
            # is sliced in halves across both rings so the stream can start
            # as soon as the first slices land.
            H = TOK_PER // 2
            xks, wks = [], []
            for k in range(K_CHUNKS):
                xk = xp.tile([128, TOK_PER], mybir.dt.bfloat16, name=f"xk{k}", tag="xk")
                wk = wp.tile([128, OUT_PER], mybir.dt.bfloat16, name=f"wk{k}", tag="wk")
                if k == 0:
                    # First pushes on each ring: the slices the very first
                    # matmuls need.  x0 first half on Scalar, w0 first half
                    # on Sync (parallel descriptor generation).
                    nc.scalar.dma_start(xk[:, 0:H], xT[0:128, 0:H])
                    nc.sync.dma_start(wk[:, 0 : OUT_PER // 2], w[0:128, 0 : OUT_PER // 2])
                    nc.sync.dma_start(xk[:, H:TOK_PER], xT[0:128, H:TOK_PER])
                    nc.scalar.dma_start(
                        wk[:, OUT_PER // 2 : OUT_PER], w[0:128, OUT_PER // 2 : OUT_PER]
                    )
                else:
                    if k % 2 == 0:
                        nc.scalar.dma_start(wk[:], w[k * 128 : (k + 1) * 128, :])
                        nc.sync.dma_start(xk[:], xT[k * 128 : (k + 1) * 128, :])
                    else:
                        nc.sync.dma_start(wk[:], w[k * 128 : (k + 1) * 128, :])
                        nc.scalar.dma_start(xk[:], xT[k * 128 : (k + 1) * 128, :])
                xks.append(xk)
                wks.append(wk)

            bias_sb = consts.tile([128, M_CHUNKS], mybir.dt.float32)
            nc.scalar.dma_start(bias_sb[:], bias.rearrange("(m p) -> p m", p=128))

            for k in range(K_CHUNKS):
                xk, wk = xks[k], wks[k]
                if k == 0:
                    # Two half-token passes so each matmul needs only the
                    # half of chunk 0 that has already landed.
                    for half in range(2):
                        for m in range(M_CHUNKS):
                            nc.tensor.matmul(
                                psums[m][:, half * H : (half + 1) * H],
                                wk[:, m * 128 : (m + 1) * 128],
                                xk[:, half * H : (half + 1) * H],
                                start=(half == 0),
                                stop=False,
                            )
                    continue
                if k >= K_CHUNKS - 3:
                    # Epilogue pipelining: run the last three chunks m-major
                    # so bank m closes ~0.65us before bank m+1 — the psum
                    # copies and output DMAs overlap the stream tail.  The
                    # last two banks close in token-halves so their copies
                    # and DMAs start half a matmul earlier.
                    if k == K_CHUNKS - 3:
                        for m in range(M_CHUNKS):
                            for kk in range(K_CHUNKS - 3, K_CHUNKS):
                                last = kk == K_CHUNKS - 1
                                if last and m == M_CHUNKS - 1:
                                    for half in range(2):
                                        nc.tensor.matmul(
                                            psums[m][:, half * H : (half + 1) * H],
                                            wks[kk][:, m * 128 : (m + 1) * 128],
                                            xks[kk][:, half * H : (half + 1) * H],
                                            start=False,
                                            stop=True,
                                        )
                                else:
                                    nc.tensor.matmul(
                                        psums[m][:],
                                        wks[kk][:, m * 128 : (m + 1) * 128],
                                        xks[kk][:],
                                        start=False,
                                        stop=last,
                                    )
                    continue
                for m in range(M_CHUNKS):
                    nc.tensor.matmul(
                        psums[m][:],
                        wk[:, m * 128 : (m + 1) * 128],
                        xk[:],
                        start=False,
                        stop=False,
                    )

            # Output: all 8 psum->sbuf copies on DVE (keeps Scalar free of
            # ACT_TABLE_LOAD at entry); one DMA push per m-chunk as each
            # copy completes, alternating rings so the tail transfer is
            # only 128 KB.
            out = op.tile([128, M_CHUNKS, TOK_PER], mybir.dt.bfloat16, bufs=1)
            for m in range(M_CHUNKS - 1):
                nc.vector.tensor_scalar_add(
                    out[:, m, :], psums[m][:], bias_sb[:, m : m + 1]
                )
                eng = nc.sync if m % 2 == 0 else nc.scalar
                eng.dma_start(
                    y[:, m * TOK_PER : (m + 1) * TOK_PER], out[:, m, :]
                )
            # Last bank: copy + DMA per token-half so the final transfers
            # are 64 KB.  The very last half-copy rides ACT (its only
            # instruction, so no queueing) in parallel with DVE's
            # second-to-last copy — the two final transfers then start
            # ~0.5us earlier on independent rings.
            m7 = M_CHUNKS - 1
            nc.vector.tensor_scalar_add(
                out[:, m7, 0:H], psums[m7][:, 0:H], bias_sb[:, m7 : m7 + 1]
            )
            nc.sync.dma_start(y[:, m7 * TOK_PER : m7 * TOK_PER + H], out[:, m7, 0:H])
            nc.scalar.activation(
                out[:, m7, H:TOK_PER],
                psums[m7][:, H:TOK_PER],
                mybir.ActivationFunctionType.Identity,
                bias=bias_sb[:, m7 : m7 + 1],
            )
            nc.scalar.dma_start(
                y[:, m7 * TOK_PER + H : (m7 + 1) * TOK_PER], out[:, m7, H:TOK_PER]
            )

    nc.finalize()
    return nc


def _densify_wT(values: np.ndarray, col_indices: np.ndarray) -> np.ndarray:
    """W^T [in=2048, out=2048] with W[r*16+i, c*16+j] = values[r,k,i,j]."""
    wT = np.zeros((C, B, R, B), dtype=np.float32)  # [c, j, r, i]
    vals_t = values.transpose(0, 1, 3, 2)  # [R, K, j, i]
    r_idx = np.arange(R)
    wT[col_indices, :, r_idx[:, None], :] = vals_t
    return wT.reshape(IN_F, OUT_F)


def kernel(x, values, col_indices, bias):
    global LAST_EXEC_TIME_NS
    import ml_dtypes

    _ensure_profile_hook()
    from concourse.bass_utils import run_bass_kernel_spmd

    if "nc" not in _CACHE:
        _CACHE["nc"] = _build_nc()
    nc = _CACHE["nc"]

    bf16 = ml_dtypes.bfloat16
    wT = _densify_wT(np.asarray(values), np.asarray(col_indices)).astype(bf16)
    xT = np.ascontiguousarray(
        np.asarray(x, dtype=np.float32).reshape(TOK, IN_F).T
    ).astype(bf16)
    bias_f = np.asarray(bias, dtype=np.float32)

    in_maps = []
    for core in range(8):
        t, h = divmod(core, OUT_SHARDS)
        in_maps.append(
            {
                "xT": np.ascontiguousarray(xT[:, t * TOK_PER : (t + 1) * TOK_PER]),
                "w": np.ascontiguousarray(wT[:, h * OUT_PER : (h + 1) * OUT_PER]),
                "bias": np.ascontiguousarray(bias_f[h * OUT_PER : (h + 1) * OUT_PER]),
            }
        )

    res = run_bass_kernel_spmd(
        nc,
        in_maps,
        list(range(8)),
        trace=bool(os.environ.get("BASS_TRACE")),
    )
    LAST_EXEC_TIME_NS = res.exec_time_ns

    y = np.empty((TOK, OUT_F), dtype=np.float32)
    for core in range(8):
        t, h = divmod(core, OUT_SHARDS)
        # [128, 8, TOK_PER] in m order: out feature = m*128 + partition
        y_dev = (
            res.results[core]["y"]
            .astype(np.float32)
            .reshape(128, M_CHUNKS, TOK_PER)
            .transpose(1, 0, 2)  # [m, p, t]
        )
        y_log = y_dev.reshape(OUT_PER, TOK_PER)
        y[t * TOK_PER : (t + 1) * TOK_PER, h * OUT_PER : (h + 1) * OUT_PER] = y_log.T
    return y.reshape(BATCH, SEQ, OUT_F)


# revision 33
# speedup vs baseline: 1.0243x; 1.0243x over previous
"""CMSBlockLinear block-ELL sparse linear forward on 8 trn2 NeuronCores.

Strategy: the block-sparse weight (R=128 x K=32 active 16x16 tiles, 25%
density) is densified on the host into W^T [2048 in, 2048 out] and cast to
bf16.  The device then runs a dense matmul y^T = W^T.T @ x^T with fp32 PSUM
accumulation.  Dense-ifying costs 4x the weight FLOPs on paper, but the PE
streams N columns per matmul regardless of M, so a dense 128-wide M uses the
array 8x better than the natural M=16 sparse formulation - dense wins on both
PE time and (with bf16) roughly matches sparse fp32 on DMA bytes.

Sharding (8 cores): 4-way over tokens x 2-way over output features.
Per core: x^T shard [2048, 512] bf16 (2 MB), W^T half [2048, 1024] bf16
(4 MB), out [1024, 512] bf16 (1 MB, upcast on host).

Device loop: k-outer over 16 contraction chunks of 128; chunks alternate
between the two HWDGE rings (Sync and Scalar) so the aggregate input
bandwidth (~270 GB/s) comfortably exceeds the PE's 150 GB/s demand.  8
dummy matmuls at the top hold the PE's HAM clock through the first-chunk
DMA wait; all 8 psum->sbuf copies ride DVE (no scalar ACT, so Scalar's
first descriptor push isn't blocked behind ACT_TABLE_LOAD); output DMAs
are pushed per-m as each copy completes, alternating rings, with the
last bank's epilogue matmuls/copies/DMAs split into token-halves so the
final 64 KB transfers' completion sems gate the exit barrier as early
as possible; the very last half-copy rides ACT (its only instruction,
so nothing queues ahead of it) in parallel with DVE's second-to-last.

Measured ~44.5-45.3 us (run-to-run spread ~0.5 us from walrus-preamble
and HAM-window phase; best observed 44522 ns).  Breakdown (from NTFF profile): the measured
window runs from the framework's const-pool memsets (~5.9 us, walrus
preamble before that is excluded) to the last exit-ladder instruction.
Fixed costs inside the window: ~1 us of preamble tail + a ~7.4 us walrus
exit ladder (each engine serially re-checks ~57 semaphores at NEFF exit;
pace is dispatch-bound — overlapping it with tail DMAs makes it slower,
measured).  The PE stream itself is ~27.3 us at the bf16 roofline
(65536 col-streams @ 2.4 GHz), starting ~11 us (first-chunk DMA
descriptor-gen + completion-sem latency) with ~0.8 us of HAM cold-clock
penalty.  fp8 (e4m3 both operands, DoubleRow) would cut PE time 1.44x
but measures 3.8% output error vs the 2% gate; int8 would pass (1.3%)
but the BIR verifier rejects integer Matmult dtypes on this toolchain.
"""

import os

import numpy as np

BATCH, SEQ = 4, 512
IN_F = OUT_F = 2048
B = 16
R = 128  # output block rows
C = 128  # input block cols
KBLK = 32  # active tiles per row

TOK = BATCH * SEQ  # 2048 tokens
TOK_SHARDS = 4
OUT_SHARDS = 2
TOK_PER = TOK // TOK_SHARDS  # 512
OUT_PER = OUT_F // OUT_SHARDS  # 1024
K_CHUNKS = IN_F // 128  # 16
M_CHUNKS = OUT_PER // 128  # 8
EPI = 4  # epilogue depth: last EPI chunks run m-major

LAST_EXEC_TIME_NS = None

_CACHE = {}


def _ensure_profile_hook():
    """Provide antenv.axon_hooks if the image lacks it, so trace=True works.

    Mirrors trn_agent_boot._ntff_profile_via_ctypes: drives NTFF capture via
    the libaxon_pjrt.so C ABI.  Also makes upload_artifacts fall back to the
    local dir when no artifact store is reachable.
    """
    import contextlib
    import ctypes
    import sys
    import types

    try:
        import antenv.axon_hooks  # noqa: F401

        return
    except ImportError:
        pass

    so_path = "/opt/axon/libaxon_pjrt.so"
    _hook = None
    if os.path.exists(so_path):
        try:
            lib = ctypes.CDLL(so_path)
            if hasattr(lib, "axon_start_nrt_profile"):
                lib.axon_start_nrt_profile.argtypes = [
                    ctypes.POINTER(ctypes.c_int64),
                    ctypes.c_size_t,
                ]
                lib.axon_start_nrt_profile.restype = ctypes.c_int64
                lib.axon_stop_nrt_profile.argtypes = [ctypes.c_char_p]
                lib.axon_stop_nrt_profile.restype = ctypes.c_int64

                @contextlib.contextmanager
                def _ntff_hook(output_dir, device_ids):
                    import jax

                    jax.devices()
                    if device_ids:
                        ids = (ctypes.c_int64 * len(device_ids))(*device_ids)
                        rc = lib.axon_start_nrt_profile(ids, len(device_ids))
                    else:
                        rc = lib.axon_start_nrt_profile(None, 0)
                    if rc != 0:
                        raise RuntimeError(f"axon_start_nrt_profile rc={rc}")
                    try:
                        yield
                    finally:
                        n = lib.axon_stop_nrt_profile(str(output_dir).encode())
                        print(f"profile: {n} file(s) -> {output_dir}", file=sys.stderr)

                _hook = _ntff_hook
        except OSError:
            pass

    mod = types.ModuleType("antenv.axon_hooks")
    mod.get_axon_ntff_profile_hook = lambda: _hook
    sys.modules["antenv.axon_hooks"] = mod

    import concourse.bass_utils as _bu

    _orig_upload = _bu.upload_artifacts

    def _safe_upload(tmpdir):
        try:
            return _orig_upload(tmpdir)
        except Exception:
            return tmpdir

    _bu.upload_artifacts = _safe_upload


def _build_nc():
    import concourse.mybir as mybir
    from concourse import bacc
    from concourse.tile import TileContext

    nc = bacc.Bacc("TRN2", target_bir_lowering=False)
    xT = nc.dram_tensor("xT", [IN_F, TOK_PER], mybir.dt.bfloat16, kind="ExternalInput")
    w = nc.dram_tensor("w", [IN_F, OUT_PER], mybir.dt.bfloat16, kind="ExternalInput")
    bias = nc.dram_tensor("bias", [OUT_PER], mybir.dt.float32, kind="ExternalInput")
    # y device layout: [partition, m-chunk, token] in m order; host un-permutes.
    y = nc.dram_tensor(
        "y", [128, M_CHUNKS * TOK_PER], mybir.dt.bfloat16, kind="ExternalOutput"
    )

    with TileContext(nc) as tc:
        with (
            tc.tile_pool(name="consts", bufs=1) as consts,
            tc.tile_pool(name="xp", bufs=K_CHUNKS) as xp,
            tc.tile_pool(name="wp", bufs=K_CHUNKS) as wp,
            tc.tile_pool(name="op", bufs=1) as op,
            tc.tile_pool(name="ps", bufs=1, space="PSUM") as ps,
        ):
            psums = [
                ps.tile([128, TOK_PER], mybir.dt.float32, tag=f"ps{m}", name=f"ps{m}")
                for m in range(M_CHUNKS)
            ]

            # HAM warm-up: dummy matmuls bridge the entry barrier (~6.9us)
            # to first-chunk data arrival (~10.3us) so the PE's activity
            # window is already hot when the real stream starts.
            warm = consts.tile([128, TOK_PER], mybir.dt.bfloat16)
            nc.vector.memset(warm[:, :1], 0)
            N_WARM = 8
            for i in range(N_WARM):
                nc.tensor.matmul(
                    psums[0][:],
                    warm[:, :128],
                    warm[:],
                    start=(i == 0),
                    stop=(i == N_WARM - 1),
                )

            # Both inputs ride both HWDGE rings: even w chunks + odd x
            # chunks on Scalar, odd w chunks + even x chunks on Sync.  Each
            # ring then carries ~(128+64) KB per 1.7us chunk period
            # (~113 GB/s), under the ~134 GB/s per-ring ceiling.  Chunk 0
            # is sliced in halves across both rings so the stream can start
            # as soon as the first slices land.
            H = TOK_PER // 2
            xks, wks = [], []
            for k in range(K_CHUNKS):
                xk = xp.tile([128, TOK_PER], mybir.dt.bfloat16, name=f"xk{k}", tag="xk")
                wk = wp.tile([128, OUT_PER], mybir.dt.bfloat16, name=f"wk{k}", tag="wk")
                if k == 0:
                    # First pushes on each ring: the slices the very first
                    # matmuls need.  x0 first half on Scalar, w0 first half
                    # on Sync (parallel descriptor generation).
                    nc.scalar.dma_start(xk[:, 0:H], xT[0:128, 0:H])
                    nc.sync.dma_start(wk[:, 0 : OUT_PER // 2], w[0:128, 0 : OUT_PER // 2])
                    nc.sync.dma_start(xk[:, H:TOK_PER], xT[0:128, H:TOK_PER])
                    nc.scalar.dma_start(
                        wk[:, OUT_PER // 2 : OUT_PER], w[0:128, OUT_PER // 2 : OUT_PER]
                    )
                else:
                    if k % 2 == 0:
                        nc.scalar.dma_start(wk[:], w[k * 128 : (k + 1) * 128, :])
                        nc.sync.dma_start(xk[:], xT[k * 128 : (k + 1) * 128, :])
                    else:
                        nc.sync.dma_start(wk[:], w[k * 128 : (k + 1) * 128, :])
                        nc.scalar.dma_start(xk[:], xT[k * 128 : (k + 1) * 128, :])
                xks.append(xk)
                wks.append(wk)

            bias_sb = consts.tile([128, M_CHUNKS], mybir.dt.float32)
            nc.scalar.dma_start(bias_sb[:], bias.rearrange("(m p) -> p m", p=128))

            for k in range(K_CHUNKS):
                xk, wk = xks[k], wks[k]
                if k == 0:
                    # Two half-token passes so each matmul needs only the
                    # half of chunk 0 that has already landed.
                    for half in range(2):
                        for m in range(M_CHUNKS):
                            nc.tensor.matmul(
                                psums[m][:, half * H : (half + 1) * H],
                                wk[:, m * 128 : (m + 1) * 128],
                                xk[:, half * H : (half + 1) * H],
                                start=(half == 0),
                                stop=False,
                            )
                    continue
                if k >= K_CHUNKS - EPI:
                    # Epilogue pipelining: run the last three chunks m-major
                    # so bank m closes ~0.65us before bank m+1 — the psum
                    # copies and output DMAs overlap the stream tail.  The
                    # last two banks close in token-halves so their copies
                    # and DMAs start half a matmul earlier.
                    if k == K_CHUNKS - EPI:
                        for m in range(M_CHUNKS):
                            for kk in range(K_CHUNKS - EPI, K_CHUNKS):
                                last = kk == K_CHUNKS - 1
                                if last and m == M_CHUNKS - 1:
                                    for half in range(2):
                                        nc.tensor.matmul(
                                            psums[m][:, half * H : (half + 1) * H],
                                            wks[kk][:, m * 128 : (m + 1) * 128],
                                            xks[kk][:, half * H : (half + 1) * H],
                                            start=False,
                                            stop=True,
                                        )
                                else:
                                    nc.tensor.matmul(
                                        psums[m][:],
                                        wks[kk][:, m * 128 : (m + 1) * 128],
                                        xks[kk][:],
                                        start=False,
                                        stop=last,
                                    )
                    continue
                for m in range(M_CHUNKS):
                    nc.tensor.matmul(
                        psums[m][:],
                        wk[:, m * 128 : (m + 1) * 128],
                        xk[:],
                        start=False,
                        stop=False,
                    )

            # Output: all 8 psum->sbuf copies on DVE (keeps Scalar free of
            # ACT_TABLE_LOAD at entry); one DMA push per m-chunk as each
            # copy completes, alternating rings so the tail transfer is
            # only 128 KB.
            out = op.tile([128, M_CHUNKS, TOK_PER], mybir.dt.bfloat16, bufs=1)
            for m in range(M_CHUNKS - 1):
                nc.vector.tensor_scalar_add(
                    out[:, m, :], psums[m][:], bias_sb[:, m : m + 1]
                )
                eng = nc.sync if m % 2 == 0 else nc.scalar
                eng.dma_start(
                    y[:, m * TOK_PER : (m + 1) * TOK_PER], out[:, m, :]
                )
            # Last bank: copy + DMA per token-half so the final transfers
            # are 64 KB.  The very last half-copy rides ACT (its only
            # instruction, so no queueing) in parallel with DVE's
            # second-to-last copy — the two final transfers then start
            # ~0.5us earlier on independent rings.
            m7 = M_CHUNKS - 1
            nc.vector.tensor_scalar_add(
                out[:, m7, 0:H], psums[m7][:, 0:H], bias_sb[:, m7 : m7 + 1]
            )
            nc.sync.dma_start(y[:, m7 * TOK_PER : m7 * TOK_PER + H], out[:, m7, 0:H])
            nc.scalar.activation(
                out[:, m7, H:TOK_PER],
                psums[m7][:, H:TOK_PER],
                mybir.ActivationFunctionType.Identity,
                bias=bias_sb[:, m7 : m7 + 1],
            )
            nc.scalar.dma_start(
                y[:, m7 * TOK_PER + H : (m7 + 1) * TOK_PER], out[:, m7, H:TOK_PER]
            )

    nc.finalize()
    return nc


def _densify_wT(values: np.ndarray, col_indices: np.ndarray) -> np.ndarray:
    """W^T [in=2048, out=2048] with W[r*16+i, c*16+j] = values[r,k,i,j]."""
    wT = np.zeros((C, B, R, B), dtype=np.float32)  # [c, j, r, i]
    vals_t = values.transpose(0, 1, 3, 2)  # [R, K, j, i]
    r_idx = np.arange(R)
    wT[col_indices, :, r_idx[:, None], :] = vals_t
    return wT.reshape(IN_F, OUT_F)


def kernel(x, values, col_indices, bias):
    global LAST_EXEC_TIME_NS
    import ml_dtypes

    _ensure_profile_hook()
    from concourse.bass_utils import run_bass_kernel_spmd

    if "nc" not in _CACHE:
        _CACHE["nc"] = _build_nc()
    nc = _CACHE["nc"]

    bf16 = ml_dtypes.bfloat16
    wT = _densify_wT(np.asarray(values), np.asarray(col_indices)).astype(bf16)
    xT = np.ascontiguousarray(
        np.asarray(x, dtype=np.float32).reshape(TOK, IN_F).T
    ).astype(bf16)
    bias_f = np.asarray(bias, dtype=np.float32)

    in_maps = []
    for core in range(8):
        t, h = divmod(core, OUT_SHARDS)
        in_maps.append(
            {
                "xT": np.ascontiguousarray(xT[:, t * TOK_PER : (t + 1) * TOK_PER]),
                "w": np.ascontiguousarray(wT[:, h * OUT_PER : (h + 1) * OUT_PER]),
                "bias": np.ascontiguousarray(bias_f[h * OUT_PER : (h + 1) * OUT_PER]),
            }
        )

    res = run_bass_kernel_spmd(
        nc,
        in_maps,
        list(range(8)),
        trace=bool(os.environ.get("BASS_TRACE")),
    )
    LAST_EXEC_TIME_NS = res.exec_time_ns

    y = np.empty((TOK, OUT_F), dtype=np.float32)
    for core in range(8):
        t, h = divmod(core, OUT_SHARDS)
        # [128, 8, TOK_PER] in m order: out feature = m*128 + partition
        y_dev = (
            res.results[core]["y"]
            .astype(np.float32)
            .reshape(128, M_CHUNKS, TOK_PER)
            .transpose(1, 0, 2)  # [m, p, t]
        )
        y_log = y_dev.reshape(OUT_PER, TOK_PER)
        y[t * TOK_PER : (t + 1) * TOK_PER, h * OUT_PER : (h + 1) * OUT_PER] = y_log.T
    return y.reshape(BATCH, SEQ, OUT_F)
